# revision 27
# baseline (speedup 1.0000x reference)
"""Trainium2 Bass kernel for nn_MultiHeadAttention (B=2, C=1024, H=16, S=2048).

Sharding: 8 cores = 2 batches x 4 head-groups (4 heads per core).
Per core:
  - Scores computed TRANSPOSED (scoresT[j,i] = k.q) in bf16. The two heads of
    a pair live in partition halves 0:64 / 64:128 of q_sb/k_sb and write the
    same [128, 2, 512] PSUM tile, so their score matmuls are emitted adjacent
    and co-issue on disjoint PE row groups (tile_position (0,0)/(64,0)).
  - One exp per (j, ib) tile on ACT ([128,1024] from PSUM). Mask applied
    multiplicatively after exp on DVE (bf16 2x mode). ctx matmul uses the
    65-col trick (vT has a ones column) for the softmax denominator.
  - PSUM: score tiles 2 banks x2 bufs + ctx accumulators 4 banks = 8 banks.
    Projection chunks recycle the score-tile pool slots so they can be
    interleaved into the attention j-loop without extra PSUM.
  - Input DMA is column-chunked and interleaved (V/Q/K chunks + mask rows) so
    attention starts ~15us in; later projection chunks (rest of vT, q/k of
    pair 1) are emitted inside the attention loop where the PE has slack.
  - Host does the final divide by the denominator row + transpose/concat.
"""

import numpy as np
import ml_dtypes

import concourse.bass as bass
import concourse.mybir as mybir
import concourse.tile as tile
from concourse import bacc
from concourse.bass_utils import run_bass_kernel_spmd

B = 2
C = 1024
HEADS = 16
CPH = 64
S = 2048
N_CORES = 8
HPC = 4  # heads per core
CPC = HPC * CPH  # channels per core = 256

BF = mybir.dt.bfloat16
F32 = mybir.dt.float32
EXP = mybir.ActivationFunctionType.Exp

NBF = ml_dtypes.bfloat16

_NC_CACHE = {}


def build_nc():
    nc = bacc.Bacc("TRN2", target_bir_lowering=False)

    Qd = nc.declare_dram_parameter("Qin", [C, S], BF, isOutput=False)
    Kd = nc.declare_dram_parameter("Kin", [C, S], BF, isOutput=False)
    Vd = nc.declare_dram_parameter("Vin", [C, S], BF, isOutput=False)
    WqTd = nc.declare_dram_parameter("WqT", [128, 8 * CPC], BF, isOutput=False)
    WkTd = nc.declare_dram_parameter("WkT", [128, 8 * CPC], BF, isOutput=False)
    WvTd = nc.declare_dram_parameter("WvT", [128, 8 * HPC * 65], BF, isOutput=False)
    bqkd = nc.declare_dram_parameter("bqk", [128, 4], F32, isOutput=False)
    bvbd = nc.declare_dram_parameter("bvb", [128, HPC * 65], F32, isOutput=False)
    Md = nc.declare_dram_parameter("maskT", [S, S], BF, isOutput=False)
    Od = nc.declare_dram_parameter("out", [HPC * 65, S], BF, isOutput=True)

    with tile.TileContext(nc) as tc:
        with (
            tc.tile_pool(name="w", bufs=1) as wp,
            tc.tile_pool(name="qksb", bufs=1) as qkp,
            tc.tile_pool(name="vt", bufs=1) as vtp,
            tc.tile_pool(name="msk", bufs=1) as mkp,
            tc.tile_pool(name="ioqk", bufs=1) as ioqk,
            tc.tile_pool(name="pt", bufs=3) as ptp,
            tc.tile_pool(name="ob", bufs=2) as obp,
            tc.tile_pool(name="sc", bufs=2, space="PSUM") as scp,
            tc.tile_pool(name="cx", bufs=2, space="PSUM") as cxp,
        ):
            # --- persistent SBUF tensors ---
            WqT = wp.tile([128, 8, CPC], BF, tag="wq")
            WkT = wp.tile([128, 8, CPC], BF, tag="wk")
            WvT = wp.tile([128, 8, HPC * 65], BF, tag="wv")
            bqk = wp.tile([128, 4], F32, tag="bqk")
            bvb = wp.tile([128, HPC * 65], F32, tag="bvb")
            dummy = wp.tile([128, 1], F32, tag="dum")
            for wt, wd in ((WqT, WqTd), (WkT, WkTd), (WvT, WvTd)):
                nc.sync.dma_start(wt[:], wd[:].rearrange("p (t n) -> p t n", t=8))
            nc.sync.dma_start(bqk[:], bqkd[:])
            nc.sync.dma_start(bvb[:], bvbd[:])
            # absorb the exp ACT_TABLE_LOAD (~2.7us) during the startup phase
            nc.scalar.activation(dummy[:], bqk[:, 0:1], EXP)

            q_sb = qkp.tile([128, 2, S], BF, tag="q")  # pair-major, even head rows 0:64
            k_sb = qkp.tile([128, 2, S], BF, tag="k")
            vT = vtp.tile([128, 16, HPC * 65], BF, tag="vt")  # s_tile-major
            maskT = mkp.tile([128, 16, S], BF, tag="m")
            Qin = ioqk.tile([128, 8, S], BF, tag="qi")
            Kin = ioqk.tile([128, 8, S], BF, tag="ki")
            Vin = ioqk.tile([128, 8, S], BF, tag="vi")

            # --- DMA emitters (1024-col halves -> 2KB DMA lines; partition
            # sub-splits spread one logical transfer over several queues) ---
            def dma_half(buf, dram, h, psplit=1):
                for ci in range(8):
                    for ps in range(psplit):
                        pr = bass.ts(ps, 128 // psplit)
                        nc.sync.dma_start(
                            buf[pr, ci, bass.ts(h, 1024)],
                            dram[
                                bass.ds(ci * 128 + ps * (128 // psplit), 128 // psplit),
                                bass.ts(h, 1024),
                            ],
                        )

            def dma_m(j):
                for ps in range(4):
                    pr = bass.ts(ps, 32)
                    nc.sync.dma_start(
                        maskT[pr, j, :], Md[bass.ds(j * 128 + ps * 32, 32), :]
                    )

            # --- projection chunk emitters (PSUM recycled from the sc pool) ---
            def proj_v(s):
                ps = scp.tile([128, 2, 512], F32, tag="sc", name="sprj")
                for ci in range(8):
                    nc.tensor.matmul(
                        ps[:, 0, : HPC * 65],
                        lhsT=Vin[:, ci, bass.ts(s, 128)],
                        rhs=WvT[:, ci, :],
                        start=(ci == 0),
                        stop=(ci == 7),
                    )
                nc.vector.tensor_add(vT[:, s, :], ps[:, 0, : HPC * 65], bvb[:])

            def proj_qk(p, qk, n4):
                dst, wt, src = (
                    (q_sb, WqT, Qin) if qk == 0 else (k_sb, WkT, Kin)
                )
                ps = scp.tile([128, 2, 512], F32, tag="sc", name="sprj")
                for ci in range(8):
                    nc.tensor.matmul(
                        ps[:, 0, :],
                        lhsT=wt[:, ci, bass.ts(p, 128)],
                        rhs=src[:, ci, bass.ts(n4, 512)],
                        start=(ci == 0),
                        stop=(ci == 7),
                    )
                nc.vector.tensor_scalar_add(
                    dst[:, p, bass.ts(n4, 512)],
                    ps[:, 0, :],
                    bqk[:, 2 * p + qk : 2 * p + qk + 1],
                )

            # --- upfront DMA; emission order sets DMA arrival order ---
            dma_half(Vin, Vd, 0, psplit=2)
            dma_half(Qin, Qd, 0, psplit=2)
            dma_half(Kin, Kd, 0, psplit=2)
            dma_m(0)
            dma_m(1)
            dma_half(Vin, Vd, 1)
            dma_m(2)
            dma_m(3)
            dma_half(Kin, Kd, 1)
            dma_m(4)
            dma_m(5)
            dma_half(Qin, Qd, 1)
            for j in range(6, 16):
                dma_m(j)

            # upfront projections (data present within the startup DMA window)
            for s in range(8):
                proj_v(s)
            for n4 in range(2):
                proj_qk(0, 0, n4)
                proj_qk(0, 1, n4)

            # interleave schedules: (p, hf) -> {j: [emit_fn, ...]}
            il = {
                (0, 0): {
                    # phase (0,0) is DMA-paced, so the PE has slack: absorb
                    # ALL remaining projections here. k chunk n4=2 feeds
                    # scores j>=8, n4=3 feeds j>=12; vT s feeds ctx j=s.
                    2: [lambda: proj_v(8)],
                    3: [lambda: proj_v(9)],
                    4: [lambda: proj_v(10)],
                    5: [lambda: proj_v(11)],
                    6: [lambda: proj_qk(0, 1, 2)],
                    7: [lambda: proj_v(12)],
                    8: [lambda: proj_v(13)],
                    9: [lambda: proj_qk(0, 1, 3)],
                    10: [lambda: proj_v(14), lambda: proj_qk(1, 0, 0)],
                    11: [lambda: proj_v(15), lambda: proj_qk(1, 1, 0)],
                    12: [lambda: proj_qk(0, 0, 2), lambda: proj_qk(1, 0, 1)],
                    13: [lambda: proj_qk(0, 0, 3), lambda: proj_qk(1, 1, 1)],
                    14: [lambda: proj_qk(1, 0, 2), lambda: proj_qk(1, 1, 2)],
                    15: [lambda: proj_qk(1, 0, 3), lambda: proj_qk(1, 1, 3)],
                },
                (0, 1): {},
                (1, 0): {},
                (1, 1): {},
            }

            # --- attention ---
            for p in range(2):
                for hf in range(2):
                    sched = il[(p, hf)]
                    cx = [
                        cxp.tile([65, 2, 512], F32, tag="cx", name=f"cx{i}")
                        for i in range(2)
                    ]
                    for j in range(16):
                        # projection chunks first: their sc-pool slot then
                        # waits on an exp that already finished last iteration,
                        # so the PE absorbs them inside the current exp window
                        for fn in sched.get(j, ()):
                            fn()
                        pts = []
                        for ib in range(2):
                            sc = scp.tile([128, 2, 512], F32, tag="sc")
                            # the two heads' score MMs co-issue on disjoint
                            # PE row groups (partitions 0:64 / 64:128)
                            for hh in range(2):
                                lo, hi = 64 * hh, 64 * hh + 64
                                nc.tensor.matmul(
                                    sc[:, hh, :],
                                    lhsT=k_sb[lo:hi, p, bass.ts(j, 128)],
                                    rhs=q_sb[
                                        lo:hi,
                                        p,
                                        bass.ds(hf * 1024 + ib * 512, 512),
                                    ],
                                    start=True,
                                    stop=True,
                                )
                            pt = ptp.tile([128, 2, 512], BF, tag="pt")
                            nc.scalar.activation(pt[:], sc[:], EXP)
                            for hh in range(2):
                                nc.vector.tensor_mul(
                                    pt[:, hh, :],
                                    pt[:, hh, :],
                                    maskT[
                                        :, j, bass.ds(hf * 1024 + ib * 512, 512)
                                    ],
                                )
                            pts.append(pt)
                        for hh in range(2):
                            hloc = 2 * p + hh
                            for ib in range(2):
                                nc.tensor.matmul(
                                    cx[hh][:, ib, :],
                                    lhsT=vT[:, j, bass.ds(hloc * 65, 65)],
                                    rhs=pts[ib][:, hh, :],
                                    start=(j == 0),
                                    stop=(j == 15),
                                )
                    # drain in half-copies split DVE/ACT so neither queue
                    # blocks the next phase's pipeline for long
                    for hh in range(2):
                        hloc = 2 * p + hh
                        ob = obp.tile([65, 2, 512], BF, tag="ob")
                        eng = nc.vector if hh == 0 else nc.scalar
                        for ib in range(2):
                            if hh == 0:
                                eng.tensor_copy(ob[:, ib, :], cx[hh][:, ib, :])
                            else:
                                eng.copy(ob[:, ib, :], cx[hh][:, ib, :])
                        nc.sync.dma_start(
                            Od[
                                bass.ds(hloc * 65, 65), bass.ts(hf, 1024)
                            ].rearrange("p (x y) -> p x y", x=2),
                            ob[:],
                        )
    nc.compile()
    return nc


def _get_nc():
    if "nc" not in _NC_CACHE:
        _NC_CACHE["nc"] = build_nc()
    return _NC_CACHE["nc"]


def _make_in_maps(Q, K, V, mask, Wq, bq, Wk, bk, Wv, bv):
    per_batch = []
    for b in range(B):
        Qa = Q[b].astype(NBF)
        Ka = K[b].astype(NBF)
        Va = V[b].astype(NBF)
        mT = np.ascontiguousarray((~mask[b]).T).astype(np.float32).astype(NBF)
        per_batch.append((Qa, Ka, Va, mT))

    in_maps = []
    for c in range(N_CORES):
        b, g = divmod(c, 4)
        hs = slice(g * CPC, (g + 1) * CPC)
        Qa, Ka, Va, mT = per_batch[b]
        # pre-rearranged to the SBUF layout [128, ci, n] so the weight DMA
        # is a single contiguous transfer (no strided descriptors)
        WqTa = np.ascontiguousarray(
            (Wq[hs].T / 8.0).reshape(8, 128, CPC).transpose(1, 0, 2).reshape(128, -1)
        ).astype(NBF)
        WkTa = np.ascontiguousarray(
            Wk[hs].T.reshape(8, 128, CPC).transpose(1, 0, 2).reshape(128, -1)
        ).astype(NBF)
        WvTa = np.zeros((C, HPC * 65), np.float32)
        bvba = np.zeros((128, HPC * 65), np.float32)
        for hh in range(HPC):
            ch = slice((g * HPC + hh) * CPH, (g * HPC + hh + 1) * CPH)
            WvTa[:, hh * 65 : hh * 65 + 64] = Wv[ch].T
            bvba[:, hh * 65 : hh * 65 + 64] = bv[ch][None, :]
            bvba[:, hh * 65 + 64] = 1.0
        # bias for q/k psum->sbuf copies: col 2p+qk = per-partition bias of
        # pair p's 128 channels (rows 0:64 = head 2p, 64:128 = head 2p+1)
        bqka = np.zeros((128, 4), np.float32)
        for p in range(2):
            ch = slice((g * 2 + p) * 128, (g * 2 + p + 1) * 128)
            bqka[:, 2 * p] = bq[ch] / 8.0
            bqka[:, 2 * p + 1] = bk[ch]
        in_maps.append(
            {
                "Qin": Qa,
                "Kin": Ka,
                "Vin": Va,
                "WqT": WqTa,
                "WkT": WkTa,
                "WvT": np.ascontiguousarray(
                    WvTa.reshape(8, 128, HPC * 65)
                    .transpose(1, 0, 2)
                    .reshape(128, -1)
                ).astype(NBF),
                "bqk": bqka,
                "bvb": bvba,
                "maskT": mT,
            }
        )
    return in_maps


def _assemble(results):
    out = np.zeros((B, S, C), np.float32)
    for c in range(N_CORES):
        b, g = divmod(c, 4)
        o = results[c]["out"].astype(np.float32)  # [260, 2048]
        for hh in range(HPC):
            ctx = o[hh * 65 : hh * 65 + 64]  # [64, S] = (d, i)
            den = o[hh * 65 + 64]  # [S]
            ch0 = (g * HPC + hh) * CPH
            out[b, :, ch0 : ch0 + CPH] = (ctx / den[None, :]).T
    return out


def run(inputs, trace=False):
    in_maps = _make_in_maps(
        np.asarray(inputs["Q"], np.float32),
        np.asarray(inputs["K"], np.float32),
        np.asarray(inputs["V"], np.float32),
        np.asarray(inputs["mask"]),
        np.asarray(inputs["Wq"], np.float32),
        np.asarray(inputs["bq"], np.float32),
        np.asarray(inputs["Wk"], np.float32),
        np.asarray(inputs["bk"], np.float32),
        np.asarray(inputs["Wv"], np.float32),
        np.asarray(inputs["bv"], np.float32),
    )
    br = run_bass_kernel_spmd(_get_nc(), in_maps, list(range(N_CORES)), trace=trace)
    return _assemble(br.results), br


def kernel(**inputs) -> np.ndarray:
    out, _ = run(inputs)
    return out


# revision 28
# speedup vs baseline: 1.1240x; 1.1240x over previous
"""Trainium2 Bass kernel for nn_MultiHeadAttention (B=2, C=1024, H=16, S=2048).

Sharding: 8 cores = 2 batches x 4 head-groups (4 heads per core).
Per core:
  - Scores computed TRANSPOSED (scoresT[j,i] = k.q) in bf16. The two heads of
    a pair live in partition halves 0:64 / 64:128 of q_sb/k_sb and write the
    same [128, 2, 512] PSUM tile, so their score matmuls are emitted adjacent
    and co-issue on disjoint PE row groups (tile_position (0,0)/(64,0)).
  - One exp per (j, ib) tile on ACT ([128,1024] from PSUM). Mask applied
    multiplicatively after exp on DVE (bf16 2x mode). ctx matmul uses the
    65-col trick (vT has a ones column) for the softmax denominator.
  - PSUM: score tiles 2 banks x2 bufs + ctx accumulators 4 banks = 8 banks.
    Projection chunks recycle the score-tile pool slots so they can be
    interleaved into the attention j-loop without extra PSUM.
  - Input DMA is column-chunked and interleaved (V/Q/K chunks + mask rows) so
    attention starts ~15us in; later projection chunks (rest of vT, q/k of
    pair 1) are emitted inside the attention loop where the PE has slack.
  - Host does the final divide by the denominator row + transpose/concat.
"""

import numpy as np
import ml_dtypes

import concourse.bass as bass
import concourse.mybir as mybir
import concourse.tile as tile
from concourse import bacc
from concourse.bass_utils import run_bass_kernel_spmd

B = 2
C = 1024
HEADS = 16
CPH = 64
S = 2048
N_CORES = 8
HPC = 4  # heads per core
CPC = HPC * CPH  # channels per core = 256

BF = mybir.dt.bfloat16
F32 = mybir.dt.float32
EXP = mybir.ActivationFunctionType.Exp

NBF = ml_dtypes.bfloat16

_NC_CACHE = {}


def build_nc():
    nc = bacc.Bacc("TRN2", target_bir_lowering=False)

    Qd = nc.declare_dram_parameter("Qin", [C, S], BF, isOutput=False)
    Kd = nc.declare_dram_parameter("Kin", [C, S], BF, isOutput=False)
    Vd = nc.declare_dram_parameter("Vin", [C, S], BF, isOutput=False)
    WqTd = nc.declare_dram_parameter("WqT", [128, 8 * CPC], BF, isOutput=False)
    WkTd = nc.declare_dram_parameter("WkT", [128, 8 * CPC], BF, isOutput=False)
    WvTd = nc.declare_dram_parameter("WvT", [128, 8 * HPC * 65], BF, isOutput=False)
    bqkd = nc.declare_dram_parameter("bqk", [128, 4], F32, isOutput=False)
    bvbd = nc.declare_dram_parameter("bvb", [128, HPC * 65], F32, isOutput=False)
    Md = nc.declare_dram_parameter("maskT", [S, S], BF, isOutput=False)
    Od = nc.declare_dram_parameter("out", [HPC * 65, S], BF, isOutput=True)

    with tile.TileContext(nc) as tc:
        with (
            tc.tile_pool(name="w", bufs=1) as wp,
            tc.tile_pool(name="qksb", bufs=1) as qkp,
            tc.tile_pool(name="vt", bufs=1) as vtp,
            tc.tile_pool(name="msk", bufs=1) as mkp,
            tc.tile_pool(name="ioqk", bufs=1) as ioqk,
            tc.tile_pool(name="pt", bufs=3) as ptp,
            tc.tile_pool(name="ob", bufs=2) as obp,
            tc.tile_pool(name="sc", bufs=2, space="PSUM") as scp,
            tc.tile_pool(name="cx", bufs=2, space="PSUM") as cxp,
        ):
            # --- persistent SBUF tensors ---
            WqT = wp.tile([128, 8, CPC], BF, tag="wq")
            WkT = wp.tile([128, 8, CPC], BF, tag="wk")
            WvT = wp.tile([128, 8, HPC * 65], BF, tag="wv")
            bqk = wp.tile([128, 4], F32, tag="bqk")
            bvb = wp.tile([128, HPC * 65], F32, tag="bvb")
            dummy = wp.tile([128, 1], F32, tag="dum")
            for wt, wd in ((WqT, WqTd), (WkT, WkTd), (WvT, WvTd)):
                nc.sync.dma_start(wt[:], wd[:].rearrange("p (t n) -> p t n", t=8))
            nc.sync.dma_start(bqk[:], bqkd[:])
            nc.sync.dma_start(bvb[:], bvbd[:])
            # absorb the exp ACT_TABLE_LOAD (~2.7us) during the startup phase
            nc.scalar.activation(dummy[:], bqk[:, 0:1], EXP)

            q_sb = qkp.tile([128, 2, S], BF, tag="q")  # pair-major, even head rows 0:64
            k_sb = qkp.tile([128, 2, S], BF, tag="k")
            vT = vtp.tile([128, 16, HPC * 65], BF, tag="vt")  # s_tile-major
            maskT = mkp.tile([128, 16, S], BF, tag="m")
            Qin = ioqk.tile([128, 8, S], BF, tag="qi")
            Kin = ioqk.tile([128, 8, S], BF, tag="ki")
            Vin = ioqk.tile([128, 8, S], BF, tag="vi")

            # --- DMA emitters (1024-col halves -> 2KB DMA lines; partition
            # sub-splits spread one logical transfer over several queues) ---
            def dma_half(buf, dram, h, psplit=1):
                for ci in range(8):
                    for ps in range(psplit):
                        pr = bass.ts(ps, 128 // psplit)
                        nc.sync.dma_start(
                            buf[pr, ci, bass.ts(h, 1024)],
                            dram[
                                bass.ds(ci * 128 + ps * (128 // psplit), 128 // psplit),
                                bass.ts(h, 1024),
                            ],
                        )

            def dma_m(j):
                for ps in range(4):
                    pr = bass.ts(ps, 32)
                    nc.sync.dma_start(
                        maskT[pr, j, :], Md[bass.ds(j * 128 + ps * 32, 32), :]
                    )

            # --- projection chunk emitters (PSUM recycled from the sc pool) ---
            def proj_v(s):
                ps = scp.tile([128, 2, 512], F32, tag="sc", name="sprj")
                for ci in range(8):
                    nc.tensor.matmul(
                        ps[:, 0, : HPC * 65],
                        lhsT=Vin[:, ci, bass.ts(s, 128)],
                        rhs=WvT[:, ci, :],
                        start=(ci == 0),
                        stop=(ci == 7),
                    )
                nc.vector.tensor_add(vT[:, s, :], ps[:, 0, : HPC * 65], bvb[:])

            def proj_qk(p, qk, n4):
                dst, wt, src = (
                    (q_sb, WqT, Qin) if qk == 0 else (k_sb, WkT, Kin)
                )
                ps = scp.tile([128, 2, 512], F32, tag="sc", name="sprj")
                for ci in range(8):
                    nc.tensor.matmul(
                        ps[:, 0, :],
                        lhsT=wt[:, ci, bass.ts(p, 128)],
                        rhs=src[:, ci, bass.ts(n4, 512)],
                        start=(ci == 0),
                        stop=(ci == 7),
                    )
                nc.vector.tensor_scalar_add(
                    dst[:, p, bass.ts(n4, 512)],
                    ps[:, 0, :],
                    bqk[:, 2 * p + qk : 2 * p + qk + 1],
                )

            # --- upfront DMA; emission order sets DMA arrival order ---
            dma_half(Vin, Vd, 0, psplit=2)
            dma_half(Qin, Qd, 0, psplit=2)
            dma_half(Kin, Kd, 0, psplit=2)
            dma_m(0)
            dma_m(1)
            dma_half(Vin, Vd, 1)
            dma_m(2)
            dma_m(3)
            dma_half(Kin, Kd, 1)
            dma_m(4)
            dma_m(5)
            dma_half(Qin, Qd, 1)
            for j in range(6, 16):
                dma_m(j)

            # upfront projections (data present within the startup DMA window)
            for s in range(8):
                proj_v(s)
            for n4 in range(2):
                proj_qk(0, 0, n4)
                proj_qk(0, 1, n4)

            # interleave schedules: (p, hf) -> {j: [emit_fn, ...]}
            il = {
                (0, 0): {
                    # k chunk n4=2 feeds scores j>=8, n4=3 feeds j>=12;
                    # one chunk per iteration so the PE window absorbs it
                    2: [lambda: proj_v(8)],
                    3: [lambda: proj_v(9)],
                    4: [lambda: proj_v(10)],
                    5: [lambda: proj_v(11)],
                    6: [lambda: proj_qk(0, 1, 2)],
                    7: [lambda: proj_v(12)],
                    8: [lambda: proj_v(13)],
                    9: [lambda: proj_qk(0, 1, 3)],
                    10: [lambda: proj_v(14)],
                    11: [lambda: proj_v(15)],
                    12: [lambda: proj_qk(0, 0, 2)],
                    13: [lambda: proj_qk(0, 0, 3)],
                },
                (0, 1): {
                    0: [lambda: proj_qk(1, 0, 0)],
                    2: [lambda: proj_qk(1, 1, 0)],
                    4: [lambda: proj_qk(1, 0, 1)],
                    6: [lambda: proj_qk(1, 1, 1)],
                    8: [lambda: proj_qk(1, 0, 2)],
                    10: [lambda: proj_qk(1, 1, 2)],
                    12: [lambda: proj_qk(1, 0, 3)],
                    14: [lambda: proj_qk(1, 1, 3)],
                },
                (1, 0): {},
                (1, 1): {},
            }

            # --- attention ---
            for p in range(2):
                for hf in range(2):
                    sched = il[(p, hf)]
                    cx = [
                        cxp.tile([65, 2, 512], F32, tag="cx", name=f"cx{i}")
                        for i in range(2)
                    ]
                    for j in range(16):
                        # projection chunks first: their sc-pool slot then
                        # waits on an exp that already finished last iteration,
                        # so the PE absorbs them inside the current exp window
                        for fn in sched.get(j, ()):
                            fn()
                        pts = []
                        for ib in range(2):
                            sc = scp.tile([128, 2, 512], F32, tag="sc")
                            # the two heads' score MMs co-issue on disjoint
                            # PE row groups (partitions 0:64 / 64:128)
                            for hh in range(2):
                                lo, hi = 64 * hh, 64 * hh + 64
                                nc.tensor.matmul(
                                    sc[:, hh, :],
                                    lhsT=k_sb[lo:hi, p, bass.ts(j, 128)],
                                    rhs=q_sb[
                                        lo:hi,
                                        p,
                                        bass.ds(hf * 1024 + ib * 512, 512),
                                    ],
                                    start=True,
                                    stop=True,
                                )
                            pt = ptp.tile([128, 2, 512], BF, tag="pt")
                            nc.scalar.activation(pt[:], sc[:], EXP)
                            for hh in range(2):
                                nc.vector.tensor_mul(
                                    pt[:, hh, :],
                                    pt[:, hh, :],
                                    maskT[
                                        :, j, bass.ds(hf * 1024 + ib * 512, 512)
                                    ],
                                )
                            pts.append(pt)
                        for hh in range(2):
                            hloc = 2 * p + hh
                            for ib in range(2):
                                nc.tensor.matmul(
                                    cx[hh][:, ib, :],
                                    lhsT=vT[:, j, bass.ds(hloc * 65, 65)],
                                    rhs=pts[ib][:, hh, :],
                                    start=(j == 0),
                                    stop=(j == 15),
                                )
                    # drain in half-copies split DVE/ACT so neither queue
                    # blocks the next phase's pipeline for long
                    for hh in range(2):
                        hloc = 2 * p + hh
                        ob = obp.tile([65, 2, 512], BF, tag="ob")
                        eng = nc.vector if hh == 0 else nc.scalar
                        for ib in range(2):
                            if hh == 0:
                                eng.tensor_copy(ob[:, ib, :], cx[hh][:, ib, :])
                            else:
                                eng.copy(ob[:, ib, :], cx[hh][:, ib, :])
                        nc.sync.dma_start(
                            Od[
                                bass.ds(hloc * 65, 65), bass.ts(hf, 1024)
                            ].rearrange("p (x y) -> p x y", x=2),
                            ob[:],
                        )
    nc.compile()
    return nc


def _get_nc():
    if "nc" not in _NC_CACHE:
        _NC_CACHE["nc"] = build_nc()
    return _NC_CACHE["nc"]


def _make_in_maps(Q, K, V, mask, Wq, bq, Wk, bk, Wv, bv):
    per_batch = []
    for b in range(B):
        Qa = Q[b].astype(NBF)
        Ka = K[b].astype(NBF)
        Va = V[b].astype(NBF)
        mT = np.ascontiguousarray((~mask[b]).T).astype(np.float32).astype(NBF)
        per_batch.append((Qa, Ka, Va, mT))

    in_maps = []
    for c in range(N_CORES):
        b, g = divmod(c, 4)
        hs = slice(g * CPC, (g + 1) * CPC)
        Qa, Ka, Va, mT = per_batch[b]
        # pre-rearranged to the SBUF layout [128, ci, n] so the weight DMA
        # is a single contiguous transfer (no strided descriptors)
        WqTa = np.ascontiguousarray(
            (Wq[hs].T / 8.0).reshape(8, 128, CPC).transpose(1, 0, 2).reshape(128, -1)
        ).astype(NBF)
        WkTa = np.ascontiguousarray(
            Wk[hs].T.reshape(8, 128, CPC).transpose(1, 0, 2).reshape(128, -1)
        ).astype(NBF)
        WvTa = np.zeros((C, HPC * 65), np.float32)
        bvba = np.zeros((128, HPC * 65), np.float32)
        for hh in range(HPC):
            ch = slice((g * HPC + hh) * CPH, (g * HPC + hh + 1) * CPH)
            WvTa[:, hh * 65 : hh * 65 + 64] = Wv[ch].T
            bvba[:, hh * 65 : hh * 65 + 64] = bv[ch][None, :]
            bvba[:, hh * 65 + 64] = 1.0
        # bias for q/k psum->sbuf copies: col 2p+qk = per-partition bias of
        # pair p's 128 channels (rows 0:64 = head 2p, 64:128 = head 2p+1)
        bqka = np.zeros((128, 4), np.float32)
        for p in range(2):
            ch = slice((g * 2 + p) * 128, (g * 2 + p + 1) * 128)
            bqka[:, 2 * p] = bq[ch] / 8.0
            bqka[:, 2 * p + 1] = bk[ch]
        in_maps.append(
            {
                "Qin": Qa,
                "Kin": Ka,
                "Vin": Va,
                "WqT": WqTa,
                "WkT": WkTa,
                "WvT": np.ascontiguousarray(
                    WvTa.reshape(8, 128, HPC * 65)
                    .transpose(1, 0, 2)
                    .reshape(128, -1)
                ).astype(NBF),
                "bqk": bqka,
                "bvb": bvba,
                "maskT": mT,
            }
        )
    return in_maps


def _assemble(results):
    out = np.zeros((B, S, C), np.float32)
    for c in range(N_CORES):
        b, g = divmod(c, 4)
        o = results[c]["out"].astype(np.float32)  # [260, 2048]
        for hh in range(HPC):
            ctx = o[hh * 65 : hh * 65 + 64]  # [64, S] = (d, i)
            den = o[hh * 65 + 64]  # [S]
            ch0 = (g * HPC + hh) * CPH
            out[b, :, ch0 : ch0 + CPH] = (ctx / den[None, :]).T
    return out


def run(inputs, trace=False):
    in_maps = _make_in_maps(
        np.asarray(inputs["Q"], np.float32),
        np.asarray(inputs["K"], np.float32),
        np.asarray(inputs["V"], np.float32),
        np.asarray(inputs["mask"]),
        np.asarray(inputs["Wq"], np.float32),
        np.asarray(inputs["bq"], np.float32),
        np.asarray(inputs["Wk"], np.float32),
        np.asarray(inputs["bk"], np.float32),
        np.asarray(inputs["Wv"], np.float32),
        np.asarray(inputs["bv"], np.float32),
    )
    br = run_bass_kernel_spmd(_get_nc(), in_maps, list(range(N_CORES)), trace=trace)
    return _assemble(br.results), br


def kernel(**inputs) -> np.ndarray:
    out, _ = run(inputs)
    return out


# revision 30
# speedup vs baseline: 1.3284x; 1.1818x over previous
"""Trainium2 Bass kernel for nn_MultiHeadAttention (B=2, C=1024, H=16, S=2048).

Sharding: 8 cores = 2 batches x 4 head-groups (4 heads per core).
Per core:
  - Scores computed TRANSPOSED (scoresT[j,i] = k.q) in bf16. The two heads of
    a pair live in partition halves 0:64 / 64:128 of q_sb/k_sb and write the
    same [128, 2, 512] PSUM tile, so their score matmuls are emitted adjacent
    and co-issue on disjoint PE row groups (tile_position (0,0)/(64,0)).
  - One exp per (j, ib) tile on ACT ([128,1024] from PSUM). Mask applied
    multiplicatively after exp on DVE (bf16 2x mode). ctx matmul uses the
    65-col trick (vT has a ones column) for the softmax denominator.
  - PSUM: score tiles 2 banks x2 bufs + ctx accumulators 4 banks = 8 banks.
    Projection chunks recycle the score-tile pool slots so they can be
    interleaved into the attention j-loop without extra PSUM.
  - Input DMA is column-chunked and interleaved (V/Q/K chunks + mask rows) so
    attention starts ~15us in; later projection chunks (rest of vT, q/k of
    pair 1) are emitted inside the attention loop where the PE has slack.
  - Host does the final divide by the denominator row + transpose/concat.
"""

import numpy as np
import ml_dtypes

import concourse.bass as bass
import concourse.mybir as mybir
import concourse.tile as tile
from concourse import bacc
from concourse.bass_utils import run_bass_kernel_spmd

B = 2
C = 1024
HEADS = 16
CPH = 64
S = 2048
N_CORES = 8
HPC = 4  # heads per core
CPC = HPC * CPH  # channels per core = 256

BF = mybir.dt.bfloat16
F32 = mybir.dt.float32
EXP = mybir.ActivationFunctionType.Exp

NBF = ml_dtypes.bfloat16

_NC_CACHE = {}


def build_nc():
    nc = bacc.Bacc("TRN2", target_bir_lowering=False)

    Qd = nc.declare_dram_parameter("Qin", [C, S], BF, isOutput=False)
    Kd = nc.declare_dram_parameter("Kin", [C, S], BF, isOutput=False)
    Vd = nc.declare_dram_parameter("Vin", [C, S], BF, isOutput=False)
    WqTd = nc.declare_dram_parameter("WqT", [128, 8 * CPC], BF, isOutput=False)
    WkTd = nc.declare_dram_parameter("WkT", [128, 8 * CPC], BF, isOutput=False)
    WvTd = nc.declare_dram_parameter("WvT", [128, 8 * HPC * 65], BF, isOutput=False)
    bqkd = nc.declare_dram_parameter("bqk", [128, 4], F32, isOutput=False)
    bvbd = nc.declare_dram_parameter("bvb", [128, HPC * 65], F32, isOutput=False)
    Md = nc.declare_dram_parameter("maskT", [S, S], BF, isOutput=False)
    Od = nc.declare_dram_parameter("out", [HPC * 65, S], BF, isOutput=True)

    with tile.TileContext(nc) as tc:
        with (
            tc.tile_pool(name="w", bufs=1) as wp,
            tc.tile_pool(name="qksb", bufs=1) as qkp,
            tc.tile_pool(name="vt", bufs=1) as vtp,
            tc.tile_pool(name="msk", bufs=1) as mkp,
            tc.tile_pool(name="ioqk", bufs=1) as ioqk,
            tc.tile_pool(name="pt", bufs=3) as ptp,
            tc.tile_pool(name="ob", bufs=2) as obp,
            tc.tile_pool(name="sc", bufs=2, space="PSUM") as scp,
            tc.tile_pool(name="cx", bufs=2, space="PSUM") as cxp,
        ):
            # --- persistent SBUF tensors ---
            WqT = wp.tile([128, 8, CPC], BF, tag="wq")
            WkT = wp.tile([128, 8, CPC], BF, tag="wk")
            WvT = wp.tile([128, 8, HPC * 65], BF, tag="wv")
            bqk = wp.tile([128, 4], F32, tag="bqk")
            bvb = wp.tile([128, HPC * 65], F32, tag="bvb")
            dummy = wp.tile([128, 1], F32, tag="dum")
            for wt, wd in ((WqT, WqTd), (WkT, WkTd), (WvT, WvTd)):
                nc.sync.dma_start(wt[:], wd[:].rearrange("p (t n) -> p t n", t=8))
            nc.sync.dma_start(bqk[:], bqkd[:])
            nc.sync.dma_start(bvb[:], bvbd[:])
            # absorb the exp ACT_TABLE_LOAD (~2.7us) during the startup phase
            nc.scalar.activation(dummy[:], bqk[:, 0:1], EXP)

            q_sb = qkp.tile([128, 2, S], BF, tag="q")  # pair-major, even head rows 0:64
            k_sb = qkp.tile([128, 2, S], BF, tag="k")
            vT = vtp.tile([128, 16, HPC * 65], BF, tag="vt")  # s_tile-major
            maskT = mkp.tile([128, 16, S], BF, tag="m")
            Qin = ioqk.tile([128, 8, S], BF, tag="qi")
            Kin = ioqk.tile([128, 8, S], BF, tag="ki")
            Vin = ioqk.tile([128, 8, S], BF, tag="vi")

            # --- DMA emitters (1024-col halves -> 2KB DMA lines) ---
            def dma_half(buf, dram, h):
                for ci in range(8):
                    nc.sync.dma_start(
                        buf[:, ci, bass.ts(h, 1024)],
                        dram[bass.ts(ci, 128), bass.ts(h, 1024)],
                    )

            def dma_m(j):
                nc.sync.dma_start(maskT[:, j, :], Md[bass.ts(j, 128), :])

            # --- projection chunk emitters (PSUM recycled from the sc pool) ---
            def proj_v(s):
                ps = scp.tile([128, 2, 512], F32, tag="sc", name="sprj")
                for ci in range(8):
                    nc.tensor.matmul(
                        ps[:, 0, : HPC * 65],
                        lhsT=Vin[:, ci, bass.ts(s, 128)],
                        rhs=WvT[:, ci, :],
                        start=(ci == 0),
                        stop=(ci == 7),
                    )
                nc.vector.tensor_add(vT[:, s, :], ps[:, 0, : HPC * 65], bvb[:])

            def proj_qk(p, qk, n4):
                dst, wt, src = (
                    (q_sb, WqT, Qin) if qk == 0 else (k_sb, WkT, Kin)
                )
                ps = scp.tile([128, 2, 512], F32, tag="sc", name="sprj")
                for ci in range(8):
                    nc.tensor.matmul(
                        ps[:, 0, :],
                        lhsT=wt[:, ci, bass.ts(p, 128)],
                        rhs=src[:, ci, bass.ts(n4, 512)],
                        start=(ci == 0),
                        stop=(ci == 7),
                    )
                nc.vector.tensor_scalar_add(
                    dst[:, p, bass.ts(n4, 512)],
                    ps[:, 0, :],
                    bqk[:, 2 * p + qk : 2 * p + qk + 1],
                )

            # --- upfront DMA; emission order sets DMA arrival order ---
            dma_half(Vin, Vd, 0)
            dma_half(Qin, Qd, 0)
            dma_half(Kin, Kd, 0)
            dma_m(0)
            dma_m(1)
            dma_half(Vin, Vd, 1)
            dma_m(2)
            dma_m(3)
            dma_half(Kin, Kd, 1)
            dma_m(4)
            dma_m(5)
            dma_half(Qin, Qd, 1)
            for j in range(6, 16):
                dma_m(j)

            # upfront projections (data present within the startup DMA window)
            for s in range(8):
                proj_v(s)
            for n4 in range(2):
                proj_qk(0, 0, n4)
                proj_qk(0, 1, n4)

            # interleave schedules: (p, hf) -> {j: [emit_fn, ...]}
            il = {
                (0, 0): {
                    # k chunk n4=2 feeds scores j>=8, n4=3 feeds j>=12;
                    # one chunk per iteration so the PE window absorbs it
                    2: [lambda: proj_v(8)],
                    3: [lambda: proj_v(9)],
                    4: [lambda: proj_v(10)],
                    5: [lambda: proj_v(11)],
                    6: [lambda: proj_qk(0, 1, 2)],
                    7: [lambda: proj_v(12)],
                    8: [lambda: proj_v(13)],
                    9: [lambda: proj_qk(0, 1, 3)],
                    10: [lambda: proj_v(14)],
                    11: [lambda: proj_v(15)],
                    12: [lambda: proj_qk(0, 0, 2)],
                    13: [lambda: proj_qk(0, 0, 3)],
                },
                (0, 1): {
                    0: [lambda: proj_qk(1, 0, 0)],
                    2: [lambda: proj_qk(1, 1, 0)],
                    4: [lambda: proj_qk(1, 0, 1)],
                    6: [lambda: proj_qk(1, 1, 1)],
                    8: [lambda: proj_qk(1, 0, 2)],
                    10: [lambda: proj_qk(1, 1, 2)],
                    12: [lambda: proj_qk(1, 0, 3)],
                    14: [lambda: proj_qk(1, 1, 3)],
                },
                (1, 0): {},
                (1, 1): {},
            }

            # --- attention ---
            for p in range(2):
                for hf in range(2):
                    sched = il[(p, hf)]
                    cx = [
                        cxp.tile([65, 2, 512], F32, tag="cx", name=f"cx{i}")
                        for i in range(2)
                    ]
                    for j in range(16):
                        # projection chunks first: their sc-pool slot then
                        # waits on an exp that already finished last iteration,
                        # so the PE absorbs them inside the current exp window
                        for fn in sched.get(j, ()):
                            fn()
                        pts = []
                        for ib in range(2):
                            sc = scp.tile([128, 2, 512], F32, tag="sc")
                            # the two heads' score MMs co-issue on disjoint
                            # PE row groups (partitions 0:64 / 64:128)
                            for hh in range(2):
                                lo, hi = 64 * hh, 64 * hh + 64
                                nc.tensor.matmul(
                                    sc[:, hh, :],
                                    lhsT=k_sb[lo:hi, p, bass.ts(j, 128)],
                                    rhs=q_sb[
                                        lo:hi,
                                        p,
                                        bass.ds(hf * 1024 + ib * 512, 512),
                                    ],
                                    start=True,
                                    stop=True,
                                )
                            pt = ptp.tile([128, 2, 512], BF, tag="pt")
                            nc.scalar.activation(pt[:], sc[:], EXP)
                            for hh in range(2):
                                nc.vector.tensor_mul(
                                    pt[:, hh, :],
                                    pt[:, hh, :],
                                    maskT[
                                        :, j, bass.ds(hf * 1024 + ib * 512, 512)
                                    ],
                                )
                            pts.append(pt)
                        for hh in range(2):
                            hloc = 2 * p + hh
                            for ib in range(2):
                                nc.tensor.matmul(
                                    cx[hh][:, ib, :],
                                    lhsT=vT[:, j, bass.ds(hloc * 65, 65)],
                                    rhs=pts[ib][:, hh, :],
                                    start=(j == 0),
                                    stop=(j == 15),
                                )
                    # drain in half-copies split DVE/ACT so neither queue
                    # blocks the next phase's pipeline for long
                    for hh in range(2):
                        hloc = 2 * p + hh
                        ob = obp.tile([65, 2, 512], BF, tag="ob")
                        eng = nc.vector if hh == 0 else nc.scalar
                        for ib in range(2):
                            if hh == 0:
                                eng.tensor_copy(ob[:, ib, :], cx[hh][:, ib, :])
                            else:
                                eng.copy(ob[:, ib, :], cx[hh][:, ib, :])
                        nc.sync.dma_start(
                            Od[
                                bass.ds(hloc * 65, 65), bass.ts(hf, 1024)
                            ].rearrange("p (x y) -> p x y", x=2),
                            ob[:],
                        )
    nc.compile()
    return nc


def _get_nc():
    if "nc" not in _NC_CACHE:
        _NC_CACHE["nc"] = build_nc()
    return _NC_CACHE["nc"]


def _make_in_maps(Q, K, V, mask, Wq, bq, Wk, bk, Wv, bv):
    per_batch = []
    for b in range(B):
        Qa = Q[b].astype(NBF)
        Ka = K[b].astype(NBF)
        Va = V[b].astype(NBF)
        mT = np.ascontiguousarray((~mask[b]).T).astype(np.float32).astype(NBF)
        per_batch.append((Qa, Ka, Va, mT))

    in_maps = []
    for c in range(N_CORES):
        b, g = divmod(c, 4)
        hs = slice(g * CPC, (g + 1) * CPC)
        Qa, Ka, Va, mT = per_batch[b]
        # pre-rearranged to the SBUF layout [128, ci, n] so the weight DMA
        # is a single contiguous transfer (no strided descriptors)
        WqTa = np.ascontiguousarray(
            (Wq[hs].T / 8.0).reshape(8, 128, CPC).transpose(1, 0, 2).reshape(128, -1)
        ).astype(NBF)
        WkTa = np.ascontiguousarray(
            Wk[hs].T.reshape(8, 128, CPC).transpose(1, 0, 2).reshape(128, -1)
        ).astype(NBF)
        WvTa = np.zeros((C, HPC * 65), np.float32)
        bvba = np.zeros((128, HPC * 65), np.float32)
        for hh in range(HPC):
            ch = slice((g * HPC + hh) * CPH, (g * HPC + hh + 1) * CPH)
            WvTa[:, hh * 65 : hh * 65 + 64] = Wv[ch].T
            bvba[:, hh * 65 : hh * 65 + 64] = bv[ch][None, :]
            bvba[:, hh * 65 + 64] = 1.0
        # bias for q/k psum->sbuf copies: col 2p+qk = per-partition bias of
        # pair p's 128 channels (rows 0:64 = head 2p, 64:128 = head 2p+1)
        bqka = np.zeros((128, 4), np.float32)
        for p in range(2):
            ch = slice((g * 2 + p) * 128, (g * 2 + p + 1) * 128)
            bqka[:, 2 * p] = bq[ch] / 8.0
            bqka[:, 2 * p + 1] = bk[ch]
        in_maps.append(
            {
                "Qin": Qa,
                "Kin": Ka,
                "Vin": Va,
                "WqT": WqTa,
                "WkT": WkTa,
                "WvT": np.ascontiguousarray(
                    WvTa.reshape(8, 128, HPC * 65)
                    .transpose(1, 0, 2)
                    .reshape(128, -1)
                ).astype(NBF),
                "bqk": bqka,
                "bvb": bvba,
                "maskT": mT,
            }
        )
    return in_maps


def _assemble(results):
    out = np.zeros((B, S, C), np.float32)
    for c in range(N_CORES):
        b, g = divmod(c, 4)
        o = results[c]["out"].astype(np.float32)  # [260, 2048]
        for hh in range(HPC):
            ctx = o[hh * 65 : hh * 65 + 64]  # [64, S] = (d, i)
            den = o[hh * 65 + 64]  # [S]
            ch0 = (g * HPC + hh) * CPH
            out[b, :, ch0 : ch0 + CPH] = (ctx / den[None, :]).T
    return out


def run(inputs, trace=False):
    in_maps = _make_in_maps(
        np.asarray(inputs["Q"], np.float32),
        np.asarray(inputs["K"], np.float32),
        np.asarray(inputs["V"], np.float32),
        np.asarray(inputs["mask"]),
        np.asarray(inputs["Wq"], np.float32),
        np.asarray(inputs["bq"], np.float32),
        np.asarray(inputs["Wk"], np.float32),
        np.asarray(inputs["bk"], np.float32),
        np.asarray(inputs["Wv"], np.float32),
        np.asarray(inputs["bv"], np.float32),
    )
    br = run_bass_kernel_spmd(_get_nc(), in_maps, list(range(N_CORES)), trace=trace)
    return _assemble(br.results), br


def kernel(**inputs) -> np.ndarray:
    out, _ = run(inputs)
    return out
